# revision 18
# baseline (speedup 1.0000x reference)
"""Trainium2 Bass kernel for nn_AttentionBlock (B=1, C=512, T=8, H=W=64).

Math: the reference's attention has seq-len 1 (softmax over a single
element == 1.0), so o == v and Q/K never affect the output:

    out = x + s(px) * (W_eff @ x)(px) + b_eff
    W_eff = w_proj @ w_v * gamma,  w_v = w_qkv[2C:3C]
    b_eff = w_proj @ b_v + b_proj
    s(px) = sqrt(C) / clip(||x[:, px]||, 1e-12)

(The per-pixel RMS scale s commutes through the channel contraction, so
the GEMM runs on raw x and s is applied to the GEMM output.)

Sharding: data-parallel over the fused (b*t)=8 frame axis, one frame per
NeuronCore; weights replicated. Per core: x [512, 4096] (channels on
partitions, pixels on the free dim).
"""

import math

import numpy as np

import concourse.tile as tile
from concourse import bacc, mybir
from concourse.bass_utils import run_bass_kernel_spmd

C = 512  # channels
T = 8  # frames == cores
PX = 4096  # pixels per frame (64*64)
NT = 512  # pixel-tile (one PSUM bank of fp32)
NTILES = PX // NT  # 8
KC = C // 128  # 4 channel chunks

F32 = mybir.dt.float32
F32R = mybir.dt.float32r

# 1e-24/C: Sqrt((sumsq + 1e-24)/C) reproduces the reference's
# clip(norm, 1e-12) for all non-degenerate inputs.
_EPS = 1e-24 / C

_BUILD_CACHE: dict = {}


def _build(has_bias: bool):
    """Trace + compile the per-core Tile program. Returns the Bacc."""
    nc = bacc.Bacc("TRN2", target_bir_lowering=False, debug=False, num_devices=T)

    # x and wt are declared float32r (same bits as f32): the PE rounds
    # fp32r operands internally, so typing the DMA chain fp32r satisfies
    # the BIR verifier with no extra conversion passes. Non-matmul
    # consumers read them through a .bitcast(F32) view.
    x = nc.dram_tensor("x", [C, PX], F32R, kind="ExternalInput").ap()
    wt = nc.dram_tensor("wt", [C, C], F32R, kind="ExternalInput").ap()
    out = nc.dram_tensor("out", [C, PX], F32, kind="ExternalOutput").ap()
    beff = None
    if has_bias:
        beff = nc.dram_tensor("beff", [1, C], F32R, kind="ExternalInput").ap()

    # [p, a, t, n]: channel-in-chunk, channel chunk, pixel tile, pixel
    xv = x.rearrange("(a p) (t n) -> p a t n", p=128, n=NT)
    ov = out.rearrange("(a p) (t n) -> p a t n", p=128, n=NT)
    # [p, a, j, m]: ci-in-chunk, ci chunk, co chunk, co-in-chunk
    wv = wt.rearrange("(a p) (j m) -> p a j m", p=128, m=128)

    with tile.TileContext(nc) as tc:
        with (
            tc.tile_pool(name="const", bufs=1) as const,
            tc.tile_pool(name="xin", bufs=5) as xin,
            tc.tile_pool(name="sq", bufs=2) as sq,
            tc.tile_pool(name="red", bufs=2) as red,
            tc.tile_pool(name="sca", bufs=2) as sca,
            tc.tile_pool(name="outp", bufs=2) as outp,
            tc.tile_pool(name="acc", bufs=3, space="PSUM") as accp,
            tc.tile_pool(name="stat", bufs=2, space="PSUM") as statp,
        ):
            ones_bf = const.tile([128, 128], F32)
            nc.vector.memset(ones_bf, 1.0)
            ones_b = const.tile([128, 128], F32R)
            nc.vector.tensor_copy(ones_b, ones_bf)
            eps_t = const.tile([128, 1], F32)
            nc.vector.memset(eps_t, _EPS)
            # weights and the first x tile land interleaved per channel
            # chunk on the sync ring (FIFO): the a=0 slices arrive in ~1.5us
            # so the first main matmuls start long before the full 2MB is
            # in. Subtile dep tracking scopes each matmul to its chunk.
            wt_sb = const.tile([128, KC, KC, 128], F32R)
            if has_bias:
                beff_sb = const.tile([1, C], F32R)
                nc.sync.dma_start(out=beff_sb, in_=beff)

            for ti in range(NTILES):
                xt = xin.tile([128, KC, NT], F32R, tag="xt")
                if ti == 0:
                    for a in range(KC):
                        nc.sync.dma_start(out=wt_sb[:, a], in_=wv[:, a])
                        nc.sync.dma_start(out=xt[:, a, :], in_=xv[:, a, ti, :])
                else:
                    nc.sync.dma_start(out=xt, in_=xv[:, :, ti, :])

                def emit_sumsq(xtu):
                    # per-pixel sum of squares over channels: square (ACT),
                    # pairwise chunk adds (GPSIMD), then ones[128,128]
                    # matmuls that reduce the partitions AND broadcast the
                    # result to every output partition.
                    x2 = sq.tile([128, KC, NT], F32, tag="x2", name="x2")
                    nc.scalar.activation(
                        out=x2,
                        in_=xtu.bitcast(F32),
                        func=mybir.ActivationFunctionType.Square,
                    )
                    xx01 = red.tile([128, NT], F32R, tag="xx01", name="xx01")
                    nc.gpsimd.tensor_add(xx01, x2[:, 0, :], x2[:, 1, :])
                    xx23 = red.tile([128, NT], F32R, tag="xx23", name="xx23")
                    nc.gpsimd.tensor_add(xx23, x2[:, 2, :], x2[:, 3, :])

                    ssb = statp.tile([128, NT], F32, tag="stat", name="ssb")
                    nc.tensor.matmul(
                        ssb, lhsT=ones_b, rhs=xx01, start=True, stop=False
                    )
                    nc.tensor.matmul(
                        ssb, lhsT=ones_b, rhs=xx23, start=False, stop=True
                    )

                    # s = 1/sqrt(sumsq/C + eps) = sqrt(C)/clip(norm, 1e-12)
                    stb = sca.tile([128, NT], F32R, tag="stb", name="stb")
                    nc.scalar.activation(
                        out=stb,
                        in_=ssb,
                        func=mybir.ActivationFunctionType.Sqrt,
                        scale=1.0 / C,
                        bias=eps_t,
                    )
                    sb_s = sca.tile([128, NT], F32, tag="sb_s", name="sb_s")
                    nc.vector.reciprocal_approx_fast(
                        out=sb_s, in_=stb.bitcast(F32)
                    )
                    return stb, sb_s

                def emit_mains(xtu):
                    accs = []
                    for jj in range(KC // 2):
                        acc = accp.tile([128, 2, NT], F32, tag="acc", name="acc")
                        accs.append(acc)
                        for q in range(2):
                            j = jj * 2 + q
                            for a in range(KC):
                                nc.tensor.matmul(
                                    acc[:, q, :],
                                    lhsT=wt_sb[:, a, j, :],
                                    rhs=xtu[:, a, :],
                                    start=(a == 0),
                                    stop=(a == KC - 1 and not has_bias),
                                )
                    return accs

                # main GEMMs first: the PE queue is in-order and the sumsq
                # matmuls wait on the ACT/GPSIMD chain, which would
                # head-of-line block the mains. Last tile: s-chain first (its
                # inputs are already pipelined in) so the combine + store
                # tail right after the final matmul instead of after it.
                if ti == NTILES - 1:
                    stb, sb_s = emit_sumsq(xt)
                    accs = emit_mains(xt)
                else:
                    accs = emit_mains(xt)
                    stb, sb_s = emit_sumsq(xt)

                if has_bias:
                    # bias enters each psum group as beff x (1/s) so the
                    # final *s yields + beff unscaled; 1/s == stb.
                    for jj in range(KC // 2):
                        for q in range(2):
                            j = jj * 2 + q
                            nc.tensor.matmul(
                                accs[jj][:, q, :],
                                lhsT=beff_sb[:, j * 128 : (j + 1) * 128],
                                rhs=stb[0:1, :],
                                start=False,
                                stop=True,
                            )

                ot = outp.tile([128, KC, NT], F32, tag="ot")
                sb_w = sb_s.unsqueeze(1).broadcast_to([128, 2, NT])
                for jj in range(KC // 2):
                    tmp = outp.tile([128, 2, NT], F32, tag="tmp", name="tmp")
                    nc.vector.tensor_mul(tmp, accs[jj], sb_w)
                    nc.vector.tensor_add(
                        ot[:, jj * 2 : jj * 2 + 2, :],
                        tmp,
                        xt[:, jj * 2 : jj * 2 + 2, :].bitcast(F32),
                    )
                    if ti == NTILES - 1:
                        # split the last store so the tail isn't one long DMA
                        nc.scalar.dma_start(
                            out=ov[:, jj * 2 : jj * 2 + 2, ti, :],
                            in_=ot[:, jj * 2 : jj * 2 + 2, :],
                        )
                if ti < NTILES - 1:
                    nc.scalar.dma_start(out=ov[:, :, ti, :], in_=ot)

    nc.compile()
    return nc


def _get_nc(has_bias: bool):
    key = has_bias
    if key not in _BUILD_CACHE:
        _BUILD_CACHE[key] = _build(has_bias)
    return _BUILD_CACHE[key]


def _prep(x, gamma, w_qkv, b_qkv, w_proj, b_proj):
    """Host-side shard + weight fold. Returns (in_maps, has_bias)."""
    x = np.asarray(x, dtype=np.float32)
    gamma = np.asarray(gamma, dtype=np.float32)
    w_qkv = np.asarray(w_qkv, dtype=np.float32)
    b_qkv = np.asarray(b_qkv, dtype=np.float32)
    w_proj = np.asarray(w_proj, dtype=np.float32)
    b_proj = np.asarray(b_proj, dtype=np.float32)

    w_v = w_qkv[2 * C : 3 * C, :]  # [cv, ci]
    b_v = b_qkv[2 * C : 3 * C]
    w_eff = (w_proj @ w_v) * gamma[None, :]  # [co, ci]
    wt = np.ascontiguousarray(w_eff.T)  # [ci, co]
    b_eff = (w_proj @ b_v + b_proj).astype(np.float32)
    has_bias = bool(np.any(b_eff != 0.0))

    in_maps = []
    for t in range(T):
        m = {
            "x": np.ascontiguousarray(x[0, :, t, :, :].reshape(C, PX)),
            "wt": wt,
        }
        if has_bias:
            m["beff"] = b_eff.reshape(1, C)
        in_maps.append(m)
    return in_maps, has_bias


def _run(inputs: dict, **run_kwargs):
    in_maps, has_bias = _prep(**inputs)
    nc = _get_nc(has_bias)
    res = run_bass_kernel_spmd(nc, in_maps, core_ids=list(range(T)), **run_kwargs)
    b, c, t, h, w = 1, C, T, 64, 64
    out = np.empty((b, c, t, h, w), dtype=np.float32)
    for i in range(T):
        out[0, :, i, :, :] = res.results[i]["out"].reshape(c, h, w)
    return out, res


def kernel(**inputs) -> np.ndarray:
    out, _ = _run(inputs)
    return out


# revision 19
# speedup vs baseline: 1.0369x; 1.0369x over previous
"""Trainium2 Bass kernel for nn_AttentionBlock (B=1, C=512, T=8, H=W=64).

Math: the reference's attention has seq-len 1 (softmax over a single
element == 1.0), so o == v and Q/K never affect the output:

    out = x + s(px) * (W_eff @ x)(px) + b_eff
    W_eff = w_proj @ w_v * gamma,  w_v = w_qkv[2C:3C]
    b_eff = w_proj @ b_v + b_proj
    s(px) = sqrt(C) / clip(||x[:, px]||, 1e-12)

(The per-pixel RMS scale s commutes through the channel contraction, so
the GEMM runs on raw x and s is applied to the GEMM output.)

Sharding: data-parallel over the fused (b*t)=8 frame axis, one frame per
NeuronCore; weights replicated. Per core: x [512, 4096] (channels on
partitions, pixels on the free dim).
"""

import math

import numpy as np

import concourse.tile as tile
from concourse import bacc, mybir
from concourse.bass_utils import run_bass_kernel_spmd

C = 512  # channels
T = 8  # frames == cores
PX = 4096  # pixels per frame (64*64)
NT = 512  # pixel-tile (one PSUM bank of fp32)
NTILES = PX // NT  # 8
KC = C // 128  # 4 channel chunks

F32 = mybir.dt.float32
F32R = mybir.dt.float32r

# 1e-24/C: Sqrt((sumsq + 1e-24)/C) reproduces the reference's
# clip(norm, 1e-12) for all non-degenerate inputs.
_EPS = 1e-24 / C

_BUILD_CACHE: dict = {}


def _build(has_bias: bool):
    """Trace + compile the per-core Tile program. Returns the Bacc."""
    nc = bacc.Bacc("TRN2", target_bir_lowering=False, debug=False, num_devices=T)

    # x and wt are declared float32r (same bits as f32): the PE rounds
    # fp32r operands internally, so typing the DMA chain fp32r satisfies
    # the BIR verifier with no extra conversion passes. Non-matmul
    # consumers read them through a .bitcast(F32) view.
    x = nc.dram_tensor("x", [C, PX], F32R, kind="ExternalInput").ap()
    wt = nc.dram_tensor("wt", [C, C], F32R, kind="ExternalInput").ap()
    out = nc.dram_tensor("out", [C, PX], F32, kind="ExternalOutput").ap()
    beff = None
    if has_bias:
        beff = nc.dram_tensor("beff", [1, C], F32R, kind="ExternalInput").ap()

    # [p, a, t, n]: channel-in-chunk, channel chunk, pixel tile, pixel
    xv = x.rearrange("(a p) (t n) -> p a t n", p=128, n=NT)
    ov = out.rearrange("(a p) (t n) -> p a t n", p=128, n=NT)
    # [p, a, j, m]: ci-in-chunk, ci chunk, co chunk, co-in-chunk
    wv = wt.rearrange("(a p) (j m) -> p a j m", p=128, m=128)

    with tile.TileContext(nc) as tc:
        with (
            tc.tile_pool(name="const", bufs=1) as const,
            tc.tile_pool(name="xin", bufs=5) as xin,
            tc.tile_pool(name="sq", bufs=2) as sq,
            tc.tile_pool(name="red", bufs=2) as red,
            tc.tile_pool(name="sca", bufs=2) as sca,
            tc.tile_pool(name="outp", bufs=2) as outp,
            tc.tile_pool(name="acc", bufs=3, space="PSUM") as accp,
            tc.tile_pool(name="stat", bufs=2, space="PSUM") as statp,
        ):
            ones_bf = const.tile([128, 128], F32)
            nc.vector.memset(ones_bf, 1.0)
            ones_b = const.tile([128, 128], F32R)
            nc.vector.tensor_copy(ones_b, ones_bf)
            eps_t = const.tile([128, 1], F32)
            nc.vector.memset(eps_t, _EPS)
            # weights and the first x tile land interleaved per channel
            # chunk on the sync ring (FIFO): the a=0 slices arrive in ~1.5us
            # so the first main matmuls start long before the full 2MB is
            # in. Subtile dep tracking scopes each matmul to its chunk.
            wt_sb = const.tile([128, KC, KC, 128], F32R)
            if has_bias:
                beff_sb = const.tile([1, C], F32R)
                nc.sync.dma_start(out=beff_sb, in_=beff)

            for ti in range(NTILES):
                xt = xin.tile([128, KC, NT], F32R, tag="xt")
                if ti == 0:
                    for a in range(KC):
                        nc.sync.dma_start(out=wt_sb[:, a], in_=wv[:, a])
                        nc.sync.dma_start(out=xt[:, a, :], in_=xv[:, a, ti, :])
                else:
                    nc.sync.dma_start(out=xt, in_=xv[:, :, ti, :])

                def emit_sumsq(xtu):
                    # per-pixel sum of squares over channels: square (ACT),
                    # pairwise chunk adds (GPSIMD), then ones[128,128]
                    # matmuls that reduce the partitions AND broadcast the
                    # result to every output partition.
                    x2 = sq.tile([128, KC, NT], F32, tag="x2", name="x2")
                    nc.scalar.activation(
                        out=x2,
                        in_=xtu.bitcast(F32),
                        func=mybir.ActivationFunctionType.Square,
                    )
                    xx01 = red.tile([128, NT], F32R, tag="xx01", name="xx01")
                    nc.gpsimd.tensor_add(xx01, x2[:, 0, :], x2[:, 1, :])
                    xx23 = red.tile([128, NT], F32R, tag="xx23", name="xx23")
                    nc.gpsimd.tensor_add(xx23, x2[:, 2, :], x2[:, 3, :])

                    ssb = statp.tile([128, NT], F32, tag="stat", name="ssb")
                    nc.tensor.matmul(
                        ssb, lhsT=ones_b, rhs=xx01, start=True, stop=False
                    )
                    nc.tensor.matmul(
                        ssb, lhsT=ones_b, rhs=xx23, start=False, stop=True
                    )

                    # s = 1/sqrt(sumsq/C + eps) = sqrt(C)/clip(norm, 1e-12)
                    stb = sca.tile([128, NT], F32R, tag="stb", name="stb")
                    nc.scalar.activation(
                        out=stb,
                        in_=ssb,
                        func=mybir.ActivationFunctionType.Sqrt,
                        scale=1.0 / C,
                        bias=eps_t,
                    )
                    sb_s = sca.tile([128, NT], F32, tag="sb_s", name="sb_s")
                    nc.vector.reciprocal_approx_fast(
                        out=sb_s, in_=stb.bitcast(F32)
                    )
                    return stb, sb_s

                def emit_mains(xtu):
                    accs = []
                    for jj in range(KC // 2):
                        acc = accp.tile([128, 2, NT], F32, tag="acc", name="acc")
                        accs.append(acc)
                        for q in range(2):
                            j = jj * 2 + q
                            for a in range(KC):
                                nc.tensor.matmul(
                                    acc[:, q, :],
                                    lhsT=wt_sb[:, a, j, :],
                                    rhs=xtu[:, a, :],
                                    start=(a == 0),
                                    stop=(a == KC - 1 and not has_bias),
                                )
                    return accs

                # main GEMMs first: the PE queue is in-order and the sumsq
                # matmuls wait on the ACT/GPSIMD chain, which would
                # head-of-line block the mains. Last tile: s-chain first (its
                # inputs are already pipelined in) so the combine + store
                # tail right after the final matmul instead of after it.
                accs = emit_mains(xt)
                stb, sb_s = emit_sumsq(xt)

                if has_bias:
                    # bias enters each psum group as beff x (1/s) so the
                    # final *s yields + beff unscaled; 1/s == stb.
                    for jj in range(KC // 2):
                        for q in range(2):
                            j = jj * 2 + q
                            nc.tensor.matmul(
                                accs[jj][:, q, :],
                                lhsT=beff_sb[:, j * 128 : (j + 1) * 128],
                                rhs=stb[0:1, :],
                                start=False,
                                stop=True,
                            )

                ot = outp.tile([128, KC, NT], F32, tag="ot")
                sb_w = sb_s.unsqueeze(1).broadcast_to([128, 2, NT])
                for jj in range(KC // 2):
                    tmp = outp.tile([128, 2, NT], F32, tag="tmp", name="tmp")
                    nc.vector.tensor_mul(tmp, accs[jj], sb_w)
                    nc.vector.tensor_add(
                        ot[:, jj * 2 : jj * 2 + 2, :],
                        tmp,
                        xt[:, jj * 2 : jj * 2 + 2, :].bitcast(F32),
                    )
                    if ti == NTILES - 1:
                        # split the last store so the tail isn't one long DMA
                        nc.scalar.dma_start(
                            out=ov[:, jj * 2 : jj * 2 + 2, ti, :],
                            in_=ot[:, jj * 2 : jj * 2 + 2, :],
                        )
                if ti < NTILES - 1:
                    nc.scalar.dma_start(out=ov[:, :, ti, :], in_=ot)

    nc.compile()
    return nc


def _get_nc(has_bias: bool):
    key = has_bias
    if key not in _BUILD_CACHE:
        _BUILD_CACHE[key] = _build(has_bias)
    return _BUILD_CACHE[key]


def _prep(x, gamma, w_qkv, b_qkv, w_proj, b_proj):
    """Host-side shard + weight fold. Returns (in_maps, has_bias)."""
    x = np.asarray(x, dtype=np.float32)
    gamma = np.asarray(gamma, dtype=np.float32)
    w_qkv = np.asarray(w_qkv, dtype=np.float32)
    b_qkv = np.asarray(b_qkv, dtype=np.float32)
    w_proj = np.asarray(w_proj, dtype=np.float32)
    b_proj = np.asarray(b_proj, dtype=np.float32)

    w_v = w_qkv[2 * C : 3 * C, :]  # [cv, ci]
    b_v = b_qkv[2 * C : 3 * C]
    w_eff = (w_proj @ w_v) * gamma[None, :]  # [co, ci]
    wt = np.ascontiguousarray(w_eff.T)  # [ci, co]
    b_eff = (w_proj @ b_v + b_proj).astype(np.float32)
    has_bias = bool(np.any(b_eff != 0.0))

    in_maps = []
    for t in range(T):
        m = {
            "x": np.ascontiguousarray(x[0, :, t, :, :].reshape(C, PX)),
            "wt": wt,
        }
        if has_bias:
            m["beff"] = b_eff.reshape(1, C)
        in_maps.append(m)
    return in_maps, has_bias


def _run(inputs: dict, **run_kwargs):
    in_maps, has_bias = _prep(**inputs)
    nc = _get_nc(has_bias)
    res = run_bass_kernel_spmd(nc, in_maps, core_ids=list(range(T)), **run_kwargs)
    b, c, t, h, w = 1, C, T, 64, 64
    out = np.empty((b, c, t, h, w), dtype=np.float32)
    for i in range(T):
        out[0, :, i, :, :] = res.results[i]["out"].reshape(c, h, w)
    return out, res


def kernel(**inputs) -> np.ndarray:
    out, _ = _run(inputs)
    return out
